# revision 107
# baseline (speedup 1.0000x reference)
"""BitNet attention layer on 8 Trainium2 NeuronCores.

Tensor-parallel over heads: core i owns heads {2i, 2i+1}. Key speed tricks:
  - QKV projection and o_proj run as fp8e4 DoubleRow matmuls (0.5 cyc/row):
    activations are split hi/lo (x = fp8(x) + fp8(x - fp8(x))) and the two
    halves ride the DoubleRow k-tile pair against stride-0-broadcast ternary
    weights, so the pair-sum reconstructs the full-precision product at 2x
    the fp32r rate with ~0.1% error.
  - attention (scores / probs / ctx / denominator) in fp16 (1 cyc/row).
  - softmax denominator via all-ones [128,128] lhsT matmul accumulation
    (partition-dim reduce + broadcast in one group).
  - o_proj drains PSUM->SBUF as plain fp16 copies (no scale); the scalar
    s_p*s_o is applied on the host after the 8 partial sums are added.
  - phases interleaved: A(st) projection+RoPE emitted oc-group-wise against
    C(t-1) o_proj sc-groups so DVE rope work and psO drains alternate.
  - wide DMAs (whole-seq-tile loads, whole-row-block stores) to amortize
    per-DMA issue (~0.6us SP.SEQ) and HWDGE (~0.6us) serialization.
Host sums the 8 partials and multiplies by s_p*s_o.
"""
import os
import sys

import numpy as np

try:
    import concourse.bass as bass
except ImportError:
    sys.path.insert(0, "/opt/trn_rl_repo")
    import concourse.bass as bass

import concourse.mybir as mybir
import concourse.tile as tile
from concourse import bacc
from concourse.bass_utils import run_bass_kernel_spmd

F32 = mybir.dt.float32
F16 = mybir.dt.float16
FP8 = mybir.dt.float8e4
DR = mybir.MatmulPerfMode.DoubleRow

S = 2048          # sequence length
H = 2048          # hidden
D = 128           # head dim
NCORES = 8
HPC = 2           # heads per core
OC = 3 * HPC * D  # 768 per-core projection output features (q|k|v)
ST = 512          # seq tile
NST = S // ST     # 4
HC = H // 128     # 16 h-chunks
HG = 4            # h-chunk group size (st0 DMA granularity)
NG = HC // HG     # 4 groups
ROPE_BASE = 10000.0

_built = None


def _build(timing=False):
    nc = bacc.Bacc("TRN2", target_bir_lowering=False, debug=False,
                   dynamic_dma_scratch_size=4096)

    if timing:
        # timing variant: identical device work, big tensors in internal DRAM
        # (garbage data) so per-call host<->device transfer is tiny.
        xh_d = nc.dram_tensor("xh_i", [H, S], FP8)
        xl_d = nc.dram_tensor("xl_i", [H, S], FP8)
        wt_d = nc.dram_tensor("wt_i", [H, OC], FP8)
        wot_d = nc.dram_tensor("wot_i", [HPC * D, H], FP8)
        cos_d = nc.dram_tensor("cost_i", [D, S], F16)
        sin_d = nc.dram_tensor("sins_i", [D, S], F16)
        tri_d = nc.dram_tensor("tri_i", [128, 896], F16)
        out_d = nc.dram_tensor("out_i", [S, H], F16)
        out_x = nc.declare_dram_parameter("out", [128, H], F16, isOutput=True)
    else:
        xh_d = nc.declare_dram_parameter("xh", [H, S], FP8, isOutput=False)
        xl_d = nc.declare_dram_parameter("xl", [H, S], FP8, isOutput=False)
        wt_d = nc.declare_dram_parameter("wt", [H, OC], FP8, isOutput=False)
        wot_d = nc.declare_dram_parameter("wot", [HPC * D, H], FP8,
                                          isOutput=False)
        cos_d = nc.declare_dram_parameter("cost", [D, S], F16, isOutput=False)
        sin_d = nc.declare_dram_parameter("sins", [D, S], F16, isOutput=False)
        tri_d = nc.declare_dram_parameter("tri", [128, 896], F16,
                                          isOutput=False)
        out_d = nc.declare_dram_parameter("out", [S, H], F16, isOutput=True)


    with tile.TileContext(nc) as tc, nc.allow_low_precision(
        reason="fp8 DoubleRow hi/lo compensated matmuls; fp16 attention"
    ):
        with tc.tile_pool(name="const", bufs=1) as cpool, \
             tc.tile_pool(name="wt", bufs=1) as wpool, \
             tc.tile_pool(name="xt", bufs=2) as xtpool, \
             tc.tile_pool(name="qkv", bufs=1) as qpool, \
             tc.tile_pool(name="pt", bufs=1) as ptpool, \
             tc.tile_pool(name="rope", bufs=2) as rpool, \
             tc.tile_pool(name="ctx", bufs=1) as xpool, \
             tc.tile_pool(name="ob", bufs=4) as opool, \
             tc.tile_pool(name="psA", bufs=2, space="PSUM") as psA, \
             tc.tile_pool(name="psS", bufs=2, space="PSUM") as psS, \
             tc.tile_pool(name="psC", bufs=1, space="PSUM") as psC, \
             tc.tile_pool(name="psO", bufs=2, space="PSUM") as psO:

            cost = cpool.tile([D, S], F16)
            sins = cpool.tile([D, S], F16)
            tri = cpool.tile([128, 896], F16)
            osq = cpool.tile([128, 128], F16)
            wot = cpool.tile([128, HPC, H], FP8)
            # all-ones lhsT for the softmax-denominator partition reduce
            nc.gpsimd.memset(osq[:], 1.0)

            # warm the PE p-state during the initial DMA feed: ~5us of
            # throwaway matmuls so real work starts at full clock
            wps = psS.tile([128, ST], F32, name="sp")
            for _ in range(34):
                nc.tensor.matmul(wps[:, 0:128], osq[:], osq[:],
                                 start=True, stop=True)

            wt = [wpool.tile([128, HG, OC], FP8, name=f"wt{g}")
                  for g in range(NG)]
            xt_tiles = {}

            def issue_xt(st):
                ssl = slice(st * ST, (st + 1) * ST)
                xts = xtpool.tile([128, HC, 2, ST], FP8, name="xts")
                nc.sync.dma_start(
                    xts[:, :, 0, :],
                    xh_d[:, ssl].rearrange("(ho hp) s -> hp ho s", hp=128))
                nc.sync.dma_start(
                    xts[:, :, 1, :],
                    xl_d[:, ssl].rearrange("(ho hp) s -> hp ho s", hp=128))
                xt_tiles[st] = xts

            # first-needed first: per-group wt/xt(st0) interleaved, RoPE
            # tables mid-feed (first RoPE runs only once the first full oc
            # accumulation completes at feed end)
            ts0 = xtpool.tile([128, HC, 2, ST], FP8, name="xts")
            xt_tiles[0] = ts0
            # startup feed alternates the two HWDGE issue queues (SP + Act)
            # so the ~0.65us/DMA issue cost pipelines 2-wide; transfers
            # stay serial on the DMA engines (the real floor)
            qs = [nc.sync, nc.scalar]
            qi = [0]

            def ldma(dst, src):
                qs[qi[0] % 2].dma_start(dst, src)
                qi[0] += 1

            for g in range(NG):
                rows = slice(g * HG * 128, (g + 1) * HG * 128)
                gho = slice(g * HG, (g + 1) * HG)
                # q/k weight columns only; the v columns load after the
                # critical startup feed (first needed by a_v, much later)
                ldma(wt[g][:, :, 0:4 * 128],
                     wt_d[rows, 0:4 * 128].rearrange("(ho hp) o -> hp ho o",
                                                     hp=128))
                ldma(ts0[:, gho, 0, :],
                     xh_d[rows, 0:ST].rearrange("(ho hp) s -> hp ho s",
                                                hp=128))
                ldma(ts0[:, gho, 1, :],
                     xl_d[rows, 0:ST].rearrange("(ho hp) s -> hp ho s",
                                                hp=128))
                if g == 1:
                    # only the st0 slice of the RoPE tables blocks A(0)
                    ldma(cost[:, 0:ST], cos_d[:, 0:ST])
                    ldma(sins[:, 0:ST], sin_d[:, 0:ST])
            for g in range(NG):
                rows = slice(g * HG * 128, (g + 1) * HG * 128)
                nc.sync.dma_start(
                    wt[g][:, :, 4 * 128:],
                    wt_d[rows, 4 * 128:].rearrange("(ho hp) o -> hp ho o",
                                                   hp=128))
            nc.sync.dma_start(tri[:], tri_d[:])
            issue_xt(1)
            nc.sync.dma_start(cost[:, ST:], cos_d[:, ST:])
            nc.sync.dma_start(sins[:, ST:], sin_d[:, ST:])
            nc.sync.dma_start(
                wot[:], wot_d.rearrange("(co cp) o -> cp co o", cp=128))

            # persistent per-head tensors
            qk = [[qpool.tile([D, ST], F16, name=f"qk{oc}_{st}")
                   for st in range(NST)] for oc in range(4)]
            v_sb = [qpool.tile([128, ST // 128, HPC * D], F16, name=f"v{st}")
                    for st in range(NST)]
            ctx = [[xpool.tile([D, 2, ST], FP8, name=f"ctx{h}_{t}")
                    for t in range(NST)] for h in range(HPC)]

            def a_qk(st, oc):
                ssl = slice(st * ST, (st + 1) * ST)
                xts = xt_tiles[st]
                ps = psA.tile([128, ST], F32)
                for hcc in range(HC):
                    g, ho = hcc // HG, hcc % HG
                    nc.tensor.matmul(
                        ps[:],
                        wt[g][:, ho, None,
                              oc * 128:(oc + 1) * 128].broadcast_to(
                                  (128, 2, 128)),
                        xts[:, hcc, :, :],
                        start=(hcc == 0), stop=(hcc == HC - 1),
                        perf_mode=DR)
                # RoPE (rotate-half sign folded into sins) -> qk f16.
                # Stage the psum through an Act fp16 copy so the DVE
                # multiplies run in 2x mode (shifts load from the saturated
                # DVE to the idler Act engine).
                dst = qk[oc][st]
                t2 = rpool.tile([128, ST], F16, name="t2")
                nc.vector.tensor_mul(t2[0:64, :], ps[64:128, :],
                                     sins[0:64, ssl])
                nc.vector.tensor_mul(t2[64:128, :], ps[0:64, :],
                                     sins[64:128, ssl])
                nc.vector.tensor_mul(dst[:], ps[:], cost[:, ssl])
                nc.vector.tensor_add(dst[:], dst[:], t2[:])

            def a_v(st, scp):
                xts = xt_tiles[st]
                ps = psA.tile([128, 2, HPC * D], F32)
                for sc2 in range(2):
                    sc = scp * 2 + sc2
                    for hcc in range(HC):
                        g, ho = hcc // HG, hcc % HG
                        nc.tensor.matmul(
                            ps[:, sc2, :],
                            xts[:, hcc, :, sc * 128:(sc + 1) * 128],
                            wt[g][:, ho, None, 4 * 128:].broadcast_to(
                                (128, 2, HPC * D)),
                            start=(hcc == 0), stop=(hcc == HC - 1),
                            perf_mode=DR)
                nc.scalar.copy(v_sb[st][:, scp * 2:scp * 2 + 2, :], ps[:])

            def phaseB(t, h, cfill=(), afill=None):
                nkj = 4 * (t + 1)
                pt = ptpool.tile([128, nkj, ST], F16, name=f"pth{h}")
                # diagonal (masked) chunks first: their Pool mask latency then
                # overlaps later exps instead of sitting at the chain tail
                jlist = list(range(4 * t, nkj)) + list(range(0, 4 * t))
                for ji, j in enumerate(jlist):
                    if h == 0 and ji == max(0, min(4, nkj - 2)):
                        # fill the exp-paced S-window with this tile's own
                        # v projection (needs only xt + wt-v, and cp needs
                        # v only after the S-loop)
                        a_v(t, 0)
                        a_v(t, 1)
                    if cfill and ji == min(10, nkj - 1):
                        for (ct_, sc_) in cfill:
                            c_sc(ct_, sc_, dve_drains=(h == 0),
                                 act3=False)
                    if afill is not None and ji == nkj // 2:
                        for oc_ in afill[1]:
                            a_qk(afill[0], oc_)
                    sp = psS.tile([128, ST], F32, name="sp")
                    nc.tensor.matmul(
                        sp[:],
                        qk[2 + h][j // 4][:, (j % 4) * 128:(j % 4 + 1) * 128],
                        qk[h][t][:],
                        start=True, stop=True)
                    # probs (unnormalized): exp(scores); score scale
                    # s_p^2/sqrt(D) lives in the RoPE tables
                    nc.scalar.activation(
                        pt[:, j, :], sp[:],
                        mybir.ActivationFunctionType.Exp,
                        bias=0.0, scale=1.0)
                    off = 128 * j - ST * t
                    if off >= 0:  # diagonal block: tril mask
                        nc.gpsimd.tensor_mul(
                            pt[:, j, :], pt[:, j, :],
                            tri[:, 384 - off:896 - off])
                # denominator first (recip overlaps ctx accumulation):
                # all-ones lhsT partition-reduce, bcast to all partitions.
                # h=0 pre-sums the chunks on DVE (fp16 2x) to save PE time;
                # h=1 keeps the full PE accumulation since its denominator
                # gates ctx -> o_proj.
                bp = psC.tile([128, ST], F32)
                if nkj > 1 and os.environ.get("BPDVE", "1") == "1":
                    # chain in jlist order: masked (diagonal) chunks first so
                    # the chain tail never waits on a Pool mask
                    pts = rpool.tile([128, ST], F16, name="pts")
                    nc.vector.tensor_add(pts[:], pt[:, jlist[0], :],
                                         pt[:, jlist[1], :])
                    for j in jlist[2:]:
                        nc.vector.tensor_add(pts[:], pts[:], pt[:, j, :])
                    nc.tensor.matmul(bp[:], osq[:], pts[:],
                                     start=True, stop=True)
                else:
                    for j in range(nkj):
                        nc.tensor.matmul(bp[:], osq[:], pt[:, j, :],
                                         start=(j == 0), stop=(j == nkj - 1))
                # ctx^T[d, qi] accumulate over kj
                cp = psC.tile([128, ST], F32)
                for j in range(nkj):
                    nc.tensor.matmul(
                        cp[:], v_sb[j // 4][:, j % 4, h * D:(h + 1) * D],
                        pt[:, j, :],
                        start=(j == 0), stop=(j == nkj - 1))
                rbp = rpool.tile([128, ST], F32, name="rbp")
                nc.vector.reciprocal(rbp[:], bp[:])
                tmp = rpool.tile([128, ST], F32, name="ctmp")
                nc.vector.tensor_mul(tmp[:], cp[:], rbp[:])
                # fp8 hi/lo split for the o_proj DoubleRow pair. Normally on
                # Pool (DVE is co-critical); for the final tile the chain
                # latency gates the endgame, so stay on DVE (no engine hops).
                ct = ctx[h][t]
                eng = nc.vector if (t == NST - 1 and h == 1) else nc.gpsimd
                eng.tensor_copy(ct[:, 0, :], tmp[:])
                eng.tensor_sub(ct[:, 1, :], tmp[:], ct[:, 0, :])

            def c_sc(t, sc, endgame=False, dve_drains=False, act3=True):
                row = (t * 4 + sc) * 128
                ob = opool.tile([128, H], F16)
                for ot in range(4):
                    if endgame:
                        # A/B phases are over: psS/psC banks are free, rotate
                        # po across all pools so drains pipeline wider
                        pool, nm = ((psO, "po"), (psS, "sp"), (psC, "bp"),
                                    (psC, "cp"))[(sc * 4 + ot) % 4]
                        po = pool.tile([128, ST], F32, name=nm)
                    else:
                        po = psO.tile([128, ST], F32, name="po")
                    for cc in range(HPC):
                        nc.tensor.matmul(
                            po[:],
                            ctx[cc][t][:, :, sc * 128:(sc + 1) * 128],
                            wot[:, cc, None,
                                ot * ST:(ot + 1) * ST].broadcast_to(
                                    (128, 2, ST)),
                            start=(cc == 0), stop=(cc == HPC - 1),
                            perf_mode=DR)
                    if (not dve_drains) and ((ot % 2 == 0) if (endgame or not act3) else (ot != 3)):
                        nc.scalar.copy(ob[:, ot * ST:(ot + 1) * ST], po[:])
                    else:
                        nc.vector.tensor_copy(ob[:, ot * ST:(ot + 1) * ST],
                                              po[:])
                    if t == NST - 1 and sc == 3:
                        # very last row block: stream half-stores so the
                        # final DMA is small
                        if ot % 2 == 1:
                            nc.sync.dma_start(
                                out_d[row:row + 128,
                                      (ot - 1) * ST:(ot + 1) * ST],
                                ob[:, (ot - 1) * ST:(ot + 1) * ST])
                if not (t == NST - 1 and sc == 3):
                    nc.sync.dma_start(out_d[row:row + 128, :], ob[:])

            def phaseA(st, cwork=(), act3=True, ocs=None):
                # interleave A(st) oc/v groups with deferred C sc groups so
                # DVE rope ops and psO drains alternate in the queues
                cs = list(cwork)
                # st0 (no C interleave): oc order 0,2,1,3 so B(0,h0)'s rope
                # gate (oc0+oc2 chains) clears two chains earlier
                order = ocs if ocs is not None else (
                    (0, 2, 1, 3) if st == 0 else (0, 1, 2, 3))
                for i, oc in enumerate(order):
                    a_qk(st, oc)
                    if i < len(cs):
                        c_sc(*cs[i], act3=act3)
                for i in range(len(order), len(cs)):
                    c_sc(*cs[i], act3=act3)
                if st + 2 < NST:
                    issue_xt(st + 2)

            # C-work is progressively deferred so the latency-bound t=3
            # region (B(3) exp pacing, ctx chains) has PE fill
            phaseA(0)
            phaseB(0, 0)
            phaseB(0, 1, afill=(1, (0, 2, 1, 3)))
            phaseA(1, cwork=[(0, 0), (0, 1), (0, 2), (0, 3)], ocs=())
            phaseB(1, 0)
            phaseB(1, 1, afill=(2, (0, 2, 1, 3)))
            phaseA(2, cwork=[(1, 0), (1, 1)], ocs=())
            phaseB(2, 0)
            phaseB(2, 1, afill=(3, (0, 2, 1, 3)))
            phaseA(3, cwork=[(1, 2), (1, 3), (2, 0)], act3=False, ocs=())
            phaseB(3, 0, cfill=[(2, 1), (2, 2)])
            phaseB(3, 1, cfill=[(2, 3)])
            for sc in range(4):
                c_sc(3, sc, endgame=True)

            if timing:
                nc.sync.dma_start(out_x[:], out_d[S - 128:, :])

    nc.compile()
    return nc


def _host_prep(hidden_states, w_proj, w_o):
    import ml_dtypes
    x = np.asarray(hidden_states, dtype=np.float32).reshape(S, H)
    w_proj = np.asarray(w_proj, dtype=np.float32)
    w_o = np.asarray(w_o, dtype=np.float32)

    # BitNet b1.58 per-tensor absmean quantization (ternary, scale factored
    # out: device weights are exactly {-1,0,1} in fp8)
    s_p = np.float32(np.mean(np.abs(w_proj), dtype=np.float32)) + np.float32(1e-5)
    s_o = np.float32(np.mean(np.abs(w_o), dtype=np.float32)) + np.float32(1e-5)
    tp = np.clip(np.round(w_proj / s_p), -1.0, 1.0).astype(np.float32)
    to = np.clip(np.round(w_o / s_o), -1.0, 1.0).astype(np.float32)

    xt = np.ascontiguousarray(x.T)                      # [H, S]
    xh = xt.astype(ml_dtypes.float8_e4m3)
    xl = (xt - xh.astype(np.float32)).astype(ml_dtypes.float8_e4m3)

    # RoPE tables, feature-major, rotate-half sign folded into sin
    inv_freq = (1.0 / (ROPE_BASE ** (np.arange(0, D, 2, dtype=np.float32) / D))
                ).astype(np.float32)
    t = np.arange(S, dtype=np.float32)
    freqs = np.outer(inv_freq, t).astype(np.float32)    # [64, S]
    # fold the score scale s_p^2/sqrt(D) into the tables (sqrt per side) so
    # the exp activation needs no per-partition scale operand
    f = np.float32(s_p / np.float32(D) ** 0.25)
    cosT = (np.concatenate([np.cos(freqs), np.cos(freqs)], 0) * f
            ).astype(np.float16)
    sinS = (np.concatenate([-np.sin(freqs), np.sin(freqs)], 0) * f
            ).astype(np.float16)

    # shifted tril mask bank: tri[p, x] = 1 if p <= x - 384
    p = np.arange(128)[:, None]
    xx = np.arange(896)[None, :]
    tri = (p <= xx - 384).astype(np.float16)

    in_maps = []
    for c in range(NCORES):
        r = slice(c * HPC * D, (c + 1) * HPC * D)       # 256 features
        wt_c = np.ascontiguousarray(np.concatenate(
            [tp[:H][r], tp[H:2 * H][r], tp[2 * H:][r]], 0).T
        ).astype(ml_dtypes.float8_e4m3)
        wot_c = np.ascontiguousarray(to[:, r].T).astype(ml_dtypes.float8_e4m3)
        in_maps.append({
            "xh": xh, "xl": xl, "wt": wt_c, "wot": wot_c,
            "cost": cosT, "sins": sinS, "tri": tri,
        })
    return in_maps, np.float32(s_p * s_o)


def kernel(hidden_states, attention_mask, w_proj, w_o):
    global _built
    if _built is None:
        _built = _build()
    nc = _built
    in_maps, osc = _host_prep(hidden_states, w_proj, w_o)
    res = run_bass_kernel_spmd(nc, in_maps, core_ids=list(range(NCORES)))
    acc = np.zeros((S, H), np.float32)
    for c in range(NCORES):
        acc += res.results[c]["out"].astype(np.float32)
    return (acc * osc).reshape(1, S, H)


# revision 116
# speedup vs baseline: 1.0225x; 1.0225x over previous
"""BitNet attention layer on 8 Trainium2 NeuronCores.

Tensor-parallel over heads: core i owns heads {2i, 2i+1}. Key speed tricks:
  - QKV projection and o_proj run as fp8e4 DoubleRow matmuls (0.5 cyc/row):
    activations are split hi/lo (x = fp8(x) + fp8(x - fp8(x))) and the two
    halves ride the DoubleRow k-tile pair against stride-0-broadcast ternary
    weights, so the pair-sum reconstructs the full-precision product at 2x
    the fp32r rate with ~0.1% error.
  - attention (scores / probs / ctx / denominator) in fp16 (1 cyc/row).
  - softmax denominator via all-ones [128,128] lhsT matmul accumulation
    (partition-dim reduce + broadcast in one group).
  - o_proj drains PSUM->SBUF as plain fp16 copies (no scale); the scalar
    s_p*s_o is applied on the host after the 8 partial sums are added.
  - phases interleaved: A(st) projection+RoPE emitted oc-group-wise against
    C(t-1) o_proj sc-groups so DVE rope work and psO drains alternate.
  - wide DMAs (whole-seq-tile loads, whole-row-block stores) to amortize
    per-DMA issue (~0.6us SP.SEQ) and HWDGE (~0.6us) serialization.
Host sums the 8 partials and multiplies by s_p*s_o.
"""
import os
import sys

import numpy as np

try:
    import concourse.bass as bass
except ImportError:
    sys.path.insert(0, "/opt/trn_rl_repo")
    import concourse.bass as bass

import concourse.mybir as mybir
import concourse.tile as tile
from concourse import bacc
from concourse.bass_utils import run_bass_kernel_spmd

F32 = mybir.dt.float32
F16 = mybir.dt.float16
FP8 = mybir.dt.float8e4
DR = mybir.MatmulPerfMode.DoubleRow

S = 2048          # sequence length
H = 2048          # hidden
D = 128           # head dim
NCORES = 8
HPC = 2           # heads per core
OC = 3 * HPC * D  # 768 per-core projection output features (q|k|v)
ST = 512          # seq tile
NST = S // ST     # 4
HC = H // 128     # 16 h-chunks
HG = 4            # h-chunk group size (st0 DMA granularity)
NG = HC // HG     # 4 groups
ROPE_BASE = 10000.0

_built = None


def _build(timing=False):
    nc = bacc.Bacc("TRN2", target_bir_lowering=False, debug=False,
                   dynamic_dma_scratch_size=4096)

    if timing:
        # timing variant: identical device work, big tensors in internal DRAM
        # (garbage data) so per-call host<->device transfer is tiny.
        xh_d = nc.dram_tensor("xh_i", [H, S], FP8)
        xl_d = nc.dram_tensor("xl_i", [H, S], FP8)
        wt_d = nc.dram_tensor("wt_i", [H, OC], FP8)
        wot_d = nc.dram_tensor("wot_i", [HPC * D, H], FP8)
        cos_d = nc.dram_tensor("cost_i", [D, S], F16)
        sin_d = nc.dram_tensor("sins_i", [D, S], F16)
        tri_d = nc.dram_tensor("tri_i", [128, 896], F16)
        out_d = nc.dram_tensor("out_i", [S, H], F16)
        out_x = nc.declare_dram_parameter("out", [128, H], F16, isOutput=True)
    else:
        xh_d = nc.declare_dram_parameter("xh", [H, S], FP8, isOutput=False)
        xl_d = nc.declare_dram_parameter("xl", [H, S], FP8, isOutput=False)
        wt_d = nc.declare_dram_parameter("wt", [H, OC], FP8, isOutput=False)
        wot_d = nc.declare_dram_parameter("wot", [HPC * D, H], FP8,
                                          isOutput=False)
        cos_d = nc.declare_dram_parameter("cost", [D, S], F16, isOutput=False)
        sin_d = nc.declare_dram_parameter("sins", [D, S], F16, isOutput=False)
        tri_d = nc.declare_dram_parameter("tri", [128, 896], F16,
                                          isOutput=False)
        out_d = nc.declare_dram_parameter("out", [S, H], F16, isOutput=True)


    with tile.TileContext(nc) as tc, nc.allow_low_precision(
        reason="fp8 DoubleRow hi/lo compensated matmuls; fp16 attention"
    ):
        with tc.tile_pool(name="const", bufs=1) as cpool, \
             tc.tile_pool(name="wt", bufs=1) as wpool, \
             tc.tile_pool(name="xt", bufs=2) as xtpool, \
             tc.tile_pool(name="qkv", bufs=1) as qpool, \
             tc.tile_pool(name="pt", bufs=1) as ptpool, \
             tc.tile_pool(name="rope", bufs=2) as rpool, \
             tc.tile_pool(name="ctx", bufs=1) as xpool, \
             tc.tile_pool(name="ob", bufs=4) as opool, \
             tc.tile_pool(name="psA", bufs=2, space="PSUM") as psA, \
             tc.tile_pool(name="psS", bufs=2, space="PSUM") as psS, \
             tc.tile_pool(name="psC", bufs=1, space="PSUM") as psC, \
             tc.tile_pool(name="psO", bufs=2, space="PSUM") as psO:

            cost = cpool.tile([D, S], F16)
            sins = cpool.tile([D, S], F16)
            tri = cpool.tile([128, 896], F16)
            osq = cpool.tile([128, 128], F16)
            wot = cpool.tile([128, HPC, H], FP8)
            # all-ones lhsT for the softmax-denominator partition reduce
            nc.gpsimd.memset(osq[:], 1.0)

            # warm the PE p-state during the initial DMA feed: ~5us of
            # throwaway matmuls so real work starts at full clock
            wps = psS.tile([128, ST], F32, name="sp")
            for _ in range(34):
                nc.tensor.matmul(wps[:, 0:128], osq[:], osq[:],
                                 start=True, stop=True)

            wt = [wpool.tile([128, HG, OC], FP8, name=f"wt{g}")
                  for g in range(NG)]
            xt_tiles = {}

            def issue_xt(st):
                ssl = slice(st * ST, (st + 1) * ST)
                xts = xtpool.tile([128, HC, 2, ST], FP8, name="xts")
                nc.sync.dma_start(
                    xts[:, :, 0, :],
                    xh_d[:, ssl].rearrange("(ho hp) s -> hp ho s", hp=128))
                nc.sync.dma_start(
                    xts[:, :, 1, :],
                    xl_d[:, ssl].rearrange("(ho hp) s -> hp ho s", hp=128))
                xt_tiles[st] = xts

            # first-needed first: per-group wt/xt(st0) interleaved, RoPE
            # tables mid-feed (first RoPE runs only once the first full oc
            # accumulation completes at feed end)
            ts0 = xtpool.tile([128, HC, 2, ST], FP8, name="xts")
            xt_tiles[0] = ts0
            # startup feed alternates the two HWDGE issue queues (SP + Act)
            # so the ~0.65us/DMA issue cost pipelines 2-wide; transfers
            # stay serial on the DMA engines (the real floor)
            qs = [nc.sync, nc.scalar]
            qi = [0]

            def ldma(dst, src):
                qs[qi[0] % 2].dma_start(dst, src)
                qi[0] += 1

            for g in range(NG):
                rows = slice(g * HG * 128, (g + 1) * HG * 128)
                gho = slice(g * HG, (g + 1) * HG)
                # q/k weight columns only; the v columns load after the
                # critical startup feed (first needed by a_v, much later)
                ldma(wt[g][:, :, 0:4 * 128],
                     wt_d[rows, 0:4 * 128].rearrange("(ho hp) o -> hp ho o",
                                                     hp=128))
                ldma(ts0[:, gho, 0, :],
                     xh_d[rows, 0:ST].rearrange("(ho hp) s -> hp ho s",
                                                hp=128))
                ldma(ts0[:, gho, 1, :],
                     xl_d[rows, 0:ST].rearrange("(ho hp) s -> hp ho s",
                                                hp=128))
                if g == 1:
                    # only the st0 slice of the RoPE tables blocks A(0)
                    ldma(cost[:, 0:ST], cos_d[:, 0:ST])
                    ldma(sins[:, 0:ST], sin_d[:, 0:ST])
            for g in range(NG):
                rows = slice(g * HG * 128, (g + 1) * HG * 128)
                nc.sync.dma_start(
                    wt[g][:, :, 4 * 128:],
                    wt_d[rows, 4 * 128:].rearrange("(ho hp) o -> hp ho o",
                                                   hp=128))
            nc.sync.dma_start(tri[:], tri_d[:])
            issue_xt(1)
            nc.sync.dma_start(cost[:, ST:], cos_d[:, ST:])
            nc.sync.dma_start(sins[:, ST:], sin_d[:, ST:])
            nc.sync.dma_start(
                wot[:], wot_d.rearrange("(co cp) o -> cp co o", cp=128))

            # persistent per-head tensors
            qk = [[qpool.tile([D, ST], F16, name=f"qk{oc}_{st}")
                   for st in range(NST)] for oc in range(4)]
            v_sb = [qpool.tile([128, ST // 128, HPC * D], F16, name=f"v{st}")
                    for st in range(NST)]
            ctx = [[xpool.tile([D, 2, ST], FP8, name=f"ctx{h}_{t}")
                    for t in range(NST)] for h in range(HPC)]

            def a_qk(st, oc):
                ssl = slice(st * ST, (st + 1) * ST)
                xts = xt_tiles[st]
                ps = psA.tile([128, ST], F32)
                for hcc in range(HC):
                    g, ho = hcc // HG, hcc % HG
                    nc.tensor.matmul(
                        ps[:],
                        wt[g][:, ho, None,
                              oc * 128:(oc + 1) * 128].broadcast_to(
                                  (128, 2, 128)),
                        xts[:, hcc, :, :],
                        start=(hcc == 0), stop=(hcc == HC - 1),
                        perf_mode=DR)
                # RoPE (rotate-half sign folded into sins) -> qk f16.
                # Stage the psum through an Act fp16 copy so the DVE
                # multiplies run in 2x mode (shifts load from the saturated
                # DVE to the idler Act engine).
                dst = qk[oc][st]
                t2 = rpool.tile([128, ST], F16, name="t2")
                nc.vector.tensor_mul(t2[0:64, :], ps[64:128, :],
                                     sins[0:64, ssl])
                nc.vector.tensor_mul(t2[64:128, :], ps[0:64, :],
                                     sins[64:128, ssl])
                nc.vector.tensor_mul(dst[:], ps[:], cost[:, ssl])
                nc.vector.tensor_add(dst[:], dst[:], t2[:])

            def a_v(st, scp):
                xts = xt_tiles[st]
                ps = psA.tile([128, 2, HPC * D], F32)
                for sc2 in range(2):
                    sc = scp * 2 + sc2
                    for hcc in range(HC):
                        g, ho = hcc // HG, hcc % HG
                        nc.tensor.matmul(
                            ps[:, sc2, :],
                            xts[:, hcc, :, sc * 128:(sc + 1) * 128],
                            wt[g][:, ho, None, 4 * 128:].broadcast_to(
                                (128, 2, HPC * D)),
                            start=(hcc == 0), stop=(hcc == HC - 1),
                            perf_mode=DR)
                nc.scalar.copy(v_sb[st][:, scp * 2:scp * 2 + 2, :], ps[:])

            def phaseB(t, h, cfill=(), afill=None):
                nkj = 4 * (t + 1)
                pt = ptpool.tile([128, nkj, ST], F16, name=f"pth{h}")
                # diagonal (masked) chunks first: their Pool mask latency then
                # overlaps later exps instead of sitting at the chain tail
                jlist = list(range(4 * t, nkj)) + list(range(0, 4 * t))
                for ji, j in enumerate(jlist):
                    if h == 0 and ji == max(0, min(4, nkj - 2)):
                        # fill the exp-paced S-window with this tile's own
                        # v projection (needs only xt + wt-v, and cp needs
                        # v only after the S-loop)
                        a_v(t, 0)
                        a_v(t, 1)
                    if cfill and ji == min(10, nkj - 1):
                        for (ct_, sc_) in cfill:
                            c_sc(ct_, sc_, dve_drains=(h == 0),
                                 act3=False)
                    if afill is not None and ji == nkj // 2:
                        for oc_ in afill[1]:
                            a_qk(afill[0], oc_)
                    sp = psS.tile([128, ST], F32, name="sp")
                    nc.tensor.matmul(
                        sp[:],
                        qk[2 + h][j // 4][:, (j % 4) * 128:(j % 4 + 1) * 128],
                        qk[h][t][:],
                        start=True, stop=True)
                    # probs (unnormalized): exp(scores); score scale
                    # s_p^2/sqrt(D) lives in the RoPE tables
                    nc.scalar.activation(
                        pt[:, j, :], sp[:],
                        mybir.ActivationFunctionType.Exp,
                        bias=0.0, scale=1.0)
                    off = 128 * j - ST * t
                    if off >= 0:  # diagonal block: tril mask
                        nc.gpsimd.tensor_mul(
                            pt[:, j, :], pt[:, j, :],
                            tri[:, 384 - off:896 - off])
                # denominator first (recip overlaps ctx accumulation):
                # all-ones lhsT partition-reduce, bcast to all partitions.
                # h=0 pre-sums the chunks on DVE (fp16 2x) to save PE time;
                # h=1 keeps the full PE accumulation since its denominator
                # gates ctx -> o_proj.
                bp = psC.tile([128, ST], F32)
                if nkj > 1 and not (t == NST - 1 and h == 1):
                    # chain in jlist order: masked (diagonal) chunks first so
                    # the chain tail never waits on a Pool mask
                    pts = rpool.tile([128, ST], F16, name="pts")
                    nc.vector.tensor_add(pts[:], pt[:, jlist[0], :],
                                         pt[:, jlist[1], :])
                    for j in jlist[2:]:
                        nc.vector.tensor_add(pts[:], pts[:], pt[:, j, :])
                    nc.tensor.matmul(bp[:], osq[:], pts[:],
                                     start=True, stop=True)
                else:
                    for j in range(nkj):
                        nc.tensor.matmul(bp[:], osq[:], pt[:, j, :],
                                         start=(j == 0), stop=(j == nkj - 1))
                # ctx^T[d, qi] accumulate over kj
                cp = psC.tile([128, ST], F32)
                for j in range(nkj):
                    nc.tensor.matmul(
                        cp[:], v_sb[j // 4][:, j % 4, h * D:(h + 1) * D],
                        pt[:, j, :],
                        start=(j == 0), stop=(j == nkj - 1))
                rbp = rpool.tile([128, ST], F32, name="rbp")
                nc.vector.reciprocal(rbp[:], bp[:])
                tmp = rpool.tile([128, ST], F32, name="ctmp")
                nc.vector.tensor_mul(tmp[:], cp[:], rbp[:])
                # fp8 hi/lo split for the o_proj DoubleRow pair. Normally on
                # Pool (DVE is co-critical); for the final tile the chain
                # latency gates the endgame, so stay on DVE (no engine hops).
                ct = ctx[h][t]
                eng = nc.vector if h == 1 else nc.gpsimd
                eng.tensor_copy(ct[:, 0, :], tmp[:])
                eng.tensor_sub(ct[:, 1, :], tmp[:], ct[:, 0, :])

            def c_sc(t, sc, endgame=False, dve_drains=False, act3=True):
                row = (t * 4 + sc) * 128
                ob = opool.tile([128, H], F16)
                for ot in range(4):
                    if endgame:
                        # A/B phases are over: psS/psC banks are free, rotate
                        # po across all pools so drains pipeline wider
                        pool, nm = ((psO, "po"), (psS, "sp"), (psC, "bp"),
                                    (psC, "cp"))[(sc * 4 + ot) % 4]
                        po = pool.tile([128, ST], F32, name=nm)
                    else:
                        po = psO.tile([128, ST], F32, name="po")
                    for cc in range(HPC):
                        nc.tensor.matmul(
                            po[:],
                            ctx[cc][t][:, :, sc * 128:(sc + 1) * 128],
                            wot[:, cc, None,
                                ot * ST:(ot + 1) * ST].broadcast_to(
                                    (128, 2, ST)),
                            start=(cc == 0), stop=(cc == HPC - 1),
                            perf_mode=DR)
                    if (not dve_drains) and ((ot % 2 == 0) if (endgame or not act3) else (ot != 3)):
                        nc.scalar.copy(ob[:, ot * ST:(ot + 1) * ST], po[:])
                    else:
                        nc.vector.tensor_copy(ob[:, ot * ST:(ot + 1) * ST],
                                              po[:])
                    if t == NST - 1 and sc == 3:
                        # very last row block: stream half-stores so the
                        # final DMA is small
                        if ot % 2 == 1:
                            nc.sync.dma_start(
                                out_d[row:row + 128,
                                      (ot - 1) * ST:(ot + 1) * ST],
                                ob[:, (ot - 1) * ST:(ot + 1) * ST])
                if not (t == NST - 1 and sc == 3):
                    nc.sync.dma_start(out_d[row:row + 128, :], ob[:])

            def phaseA(st, cwork=(), act3=True, ocs=None):
                # interleave A(st) oc/v groups with deferred C sc groups so
                # DVE rope ops and psO drains alternate in the queues
                cs = list(cwork)
                # st0 (no C interleave): oc order 0,2,1,3 so B(0,h0)'s rope
                # gate (oc0+oc2 chains) clears two chains earlier
                order = ocs if ocs is not None else (
                    (0, 2, 1, 3) if st == 0 else (0, 1, 2, 3))
                for i, oc in enumerate(order):
                    a_qk(st, oc)
                    if i < len(cs):
                        c_sc(*cs[i], act3=act3)
                for i in range(len(order), len(cs)):
                    c_sc(*cs[i], act3=act3)
                if st + 2 < NST:
                    issue_xt(st + 2)

            # C-work is progressively deferred so the latency-bound t=3
            # region (B(3) exp pacing, ctx chains) has PE fill
            phaseA(0)
            phaseB(0, 0)
            phaseB(0, 1, afill=(1, (0, 2, 1, 3)))
            phaseA(1, cwork=[(0, 0), (0, 1), (0, 2), (0, 3)], ocs=())
            phaseB(1, 0)
            phaseB(1, 1, afill=(2, (0, 2, 1, 3)))
            phaseA(2, cwork=[(1, 0), (1, 1)], ocs=())
            phaseB(2, 0)
            phaseB(2, 1, afill=(3, (0, 2, 1, 3)))
            phaseA(3, cwork=[(1, 2), (1, 3), (2, 0)], act3=False, ocs=())
            phaseB(3, 0, cfill=[(2, 1), (2, 2)])
            phaseB(3, 1, cfill=[(2, 3)])
            for sc in range(4):
                c_sc(3, sc, endgame=True)

            if timing:
                nc.sync.dma_start(out_x[:], out_d[S - 128:, :])

    nc.compile()
    return nc


def _host_prep(hidden_states, w_proj, w_o):
    import ml_dtypes
    x = np.asarray(hidden_states, dtype=np.float32).reshape(S, H)
    w_proj = np.asarray(w_proj, dtype=np.float32)
    w_o = np.asarray(w_o, dtype=np.float32)

    # BitNet b1.58 per-tensor absmean quantization (ternary, scale factored
    # out: device weights are exactly {-1,0,1} in fp8)
    s_p = np.float32(np.mean(np.abs(w_proj), dtype=np.float32)) + np.float32(1e-5)
    s_o = np.float32(np.mean(np.abs(w_o), dtype=np.float32)) + np.float32(1e-5)
    tp = np.clip(np.round(w_proj / s_p), -1.0, 1.0).astype(np.float32)
    to = np.clip(np.round(w_o / s_o), -1.0, 1.0).astype(np.float32)

    xt = np.ascontiguousarray(x.T)                      # [H, S]
    xh = xt.astype(ml_dtypes.float8_e4m3)
    xl = (xt - xh.astype(np.float32)).astype(ml_dtypes.float8_e4m3)

    # RoPE tables, feature-major, rotate-half sign folded into sin
    inv_freq = (1.0 / (ROPE_BASE ** (np.arange(0, D, 2, dtype=np.float32) / D))
                ).astype(np.float32)
    t = np.arange(S, dtype=np.float32)
    freqs = np.outer(inv_freq, t).astype(np.float32)    # [64, S]
    # fold the score scale s_p^2/sqrt(D) into the tables (sqrt per side) so
    # the exp activation needs no per-partition scale operand
    f = np.float32(s_p / np.float32(D) ** 0.25)
    cosT = (np.concatenate([np.cos(freqs), np.cos(freqs)], 0) * f
            ).astype(np.float16)
    sinS = (np.concatenate([-np.sin(freqs), np.sin(freqs)], 0) * f
            ).astype(np.float16)

    # shifted tril mask bank: tri[p, x] = 1 if p <= x - 384
    p = np.arange(128)[:, None]
    xx = np.arange(896)[None, :]
    tri = (p <= xx - 384).astype(np.float16)

    in_maps = []
    for c in range(NCORES):
        r = slice(c * HPC * D, (c + 1) * HPC * D)       # 256 features
        wt_c = np.ascontiguousarray(np.concatenate(
            [tp[:H][r], tp[H:2 * H][r], tp[2 * H:][r]], 0).T
        ).astype(ml_dtypes.float8_e4m3)
        wot_c = np.ascontiguousarray(to[:, r].T).astype(ml_dtypes.float8_e4m3)
        in_maps.append({
            "xh": xh, "xl": xl, "wt": wt_c, "wot": wot_c,
            "cost": cosT, "sins": sinS, "tri": tri,
        })
    return in_maps, np.float32(s_p * s_o)


def kernel(hidden_states, attention_mask, w_proj, w_o):
    global _built
    if _built is None:
        _built = _build()
    nc = _built
    in_maps, osc = _host_prep(hidden_states, w_proj, w_o)
    res = run_bass_kernel_spmd(nc, in_maps, core_ids=list(range(NCORES)))
    acc = np.zeros((S, H), np.float32)
    for c in range(NCORES):
        acc += res.results[c]["out"].astype(np.float32)
    return (acc * osc).reshape(1, S, H)


# revision 131
# speedup vs baseline: 1.0452x; 1.0222x over previous
"""BitNet attention layer on 8 Trainium2 NeuronCores.

Tensor-parallel over heads: core i owns heads {2i, 2i+1}. Key speed tricks:
  - QKV projection and o_proj run as fp8e4 DoubleRow matmuls (0.5 cyc/row):
    activations are split hi/lo (x = fp8(x) + fp8(x - fp8(x))) and the two
    halves ride the DoubleRow k-tile pair against stride-0-broadcast ternary
    weights, so the pair-sum reconstructs the full-precision product at 2x
    the fp32r rate with ~0.1% error.
  - attention (scores / probs / ctx / denominator) in fp16 (1 cyc/row).
  - softmax denominator via all-ones [128,128] lhsT matmul accumulation
    (partition-dim reduce + broadcast in one group).
  - o_proj drains PSUM->SBUF as plain fp16 copies (no scale); the scalar
    s_p*s_o is applied on the host after the 8 partial sums are added.
  - phases interleaved: A(st) projection+RoPE emitted oc-group-wise against
    C(t-1) o_proj sc-groups so DVE rope work and psO drains alternate.
  - wide DMAs (whole-seq-tile loads, whole-row-block stores) to amortize
    per-DMA issue (~0.6us SP.SEQ) and HWDGE (~0.6us) serialization.
Host sums the 8 partials and multiplies by s_p*s_o.
"""
import os
import sys

import numpy as np

try:
    import concourse.bass as bass
except ImportError:
    sys.path.insert(0, "/opt/trn_rl_repo")
    import concourse.bass as bass

import concourse.mybir as mybir
import concourse.tile as tile
from concourse import bacc
from concourse.bass_utils import run_bass_kernel_spmd

F32 = mybir.dt.float32
F16 = mybir.dt.float16
FP8 = mybir.dt.float8e4
DR = mybir.MatmulPerfMode.DoubleRow

S = 2048          # sequence length
H = 2048          # hidden
D = 128           # head dim
NCORES = 8
HPC = 2           # heads per core
OC = 3 * HPC * D  # 768 per-core projection output features (q|k|v)
ST = 512          # seq tile
NST = S // ST     # 4
HC = H // 128     # 16 h-chunks
HG = 4            # h-chunk group size (st0 DMA granularity)
NG = HC // HG     # 4 groups
ROPE_BASE = 10000.0

_built = None


def _build(timing=False):
    nc = bacc.Bacc("TRN2", target_bir_lowering=False, debug=False,
                   dynamic_dma_scratch_size=4096)

    if timing:
        # timing variant: identical device work, big tensors in internal DRAM
        # (garbage data) so per-call host<->device transfer is tiny.
        xh_d = nc.dram_tensor("xh_i", [H, S], FP8)
        xl_d = nc.dram_tensor("xl_i", [H, S], FP8)
        wt_d = nc.dram_tensor("wt_i", [H, OC], FP8)
        wot_d = nc.dram_tensor("wot_i", [HPC * D, H], FP8)
        cos_d = nc.dram_tensor("cost_i", [D, S], F16)
        sin_d = nc.dram_tensor("sins_i", [D, S], F16)
        tri_d = nc.dram_tensor("tri_i", [128, 896], F16)
        out_d = nc.dram_tensor("out_i", [S, H], F16)
        out_x = nc.declare_dram_parameter("out", [128, H], F16, isOutput=True)
    else:
        xh_d = nc.declare_dram_parameter("xh", [H, S], FP8, isOutput=False)
        xl_d = nc.declare_dram_parameter("xl", [H, S], FP8, isOutput=False)
        wt_d = nc.declare_dram_parameter("wt", [H, OC], FP8, isOutput=False)
        wot_d = nc.declare_dram_parameter("wot", [HPC * D, H], FP8,
                                          isOutput=False)
        cos_d = nc.declare_dram_parameter("cost", [D, S], F16, isOutput=False)
        sin_d = nc.declare_dram_parameter("sins", [D, S], F16, isOutput=False)
        tri_d = nc.declare_dram_parameter("tri", [128, 896], F16,
                                          isOutput=False)
        out_d = nc.declare_dram_parameter("out", [S, H], F16, isOutput=True)


    with tile.TileContext(nc) as tc, nc.allow_low_precision(
        reason="fp8 DoubleRow hi/lo compensated matmuls; fp16 attention"
    ):
        with tc.tile_pool(name="const", bufs=1) as cpool, \
             tc.tile_pool(name="wt", bufs=1) as wpool, \
             tc.tile_pool(name="xt", bufs=2) as xtpool, \
             tc.tile_pool(name="qkv", bufs=1) as qpool, \
             tc.tile_pool(name="pt", bufs=1) as ptpool, \
             tc.tile_pool(name="rope", bufs=2) as rpool, \
             tc.tile_pool(name="ctx", bufs=1) as xpool, \
             tc.tile_pool(name="ob", bufs=4) as opool, \
             tc.tile_pool(name="psA", bufs=2, space="PSUM") as psA, \
             tc.tile_pool(name="psS", bufs=2, space="PSUM") as psS, \
             tc.tile_pool(name="psC", bufs=1, space="PSUM") as psC, \
             tc.tile_pool(name="psO", bufs=2, space="PSUM") as psO:

            cost = cpool.tile([D, S], F16)
            sins = cpool.tile([D, S], F16)
            tri = cpool.tile([128, 896], F16)
            osq = cpool.tile([128, 128], F16)
            wot = cpool.tile([128, HPC, H], FP8)
            # all-ones lhsT for the softmax-denominator partition reduce
            nc.gpsimd.memset(osq[:], 1.0)

            # warm the PE p-state during the initial DMA feed: ~5us of
            # throwaway matmuls so real work starts at full clock
            wps = psS.tile([128, ST], F32, name="sp")
            for _ in range(34):
                nc.tensor.matmul(wps[:, 0:128], osq[:], osq[:],
                                 start=True, stop=True)

            wt = [wpool.tile([128, HG, OC], FP8, name=f"wt{g}")
                  for g in range(NG)]
            xt_tiles = {}

            def issue_xt(st):
                ssl = slice(st * ST, (st + 1) * ST)
                xts = xtpool.tile([128, HC, 2, ST], FP8, name="xts")
                nc.sync.dma_start(
                    xts[:, :, 0, :],
                    xh_d[:, ssl].rearrange("(ho hp) s -> hp ho s", hp=128))
                nc.sync.dma_start(
                    xts[:, :, 1, :],
                    xl_d[:, ssl].rearrange("(ho hp) s -> hp ho s", hp=128))
                xt_tiles[st] = xts

            # first-needed first: per-group wt/xt(st0) interleaved, RoPE
            # tables mid-feed (first RoPE runs only once the first full oc
            # accumulation completes at feed end)
            ts0 = xtpool.tile([128, HC, 2, ST], FP8, name="xts")
            xt_tiles[0] = ts0
            # startup feed alternates the two HWDGE issue queues (SP + Act)
            # so the ~0.65us/DMA issue cost pipelines 2-wide; transfers
            # stay serial on the DMA engines (the real floor)
            qs = [nc.sync, nc.scalar]
            qi = [0]

            def ldma(dst, src):
                qs[qi[0] % 2].dma_start(dst, src)
                qi[0] += 1

            for g in range(NG):
                rows = slice(g * HG * 128, (g + 1) * HG * 128)
                gho = slice(g * HG, (g + 1) * HG)
                # q/k weight columns only; the v columns load after the
                # critical startup feed (first needed by a_v, much later)
                ldma(wt[g][:, :, 0:4 * 128],
                     wt_d[rows, 0:4 * 128].rearrange("(ho hp) o -> hp ho o",
                                                     hp=128))
                ldma(ts0[:, gho, 0, :],
                     xh_d[rows, 0:ST].rearrange("(ho hp) s -> hp ho s",
                                                hp=128))
                ldma(ts0[:, gho, 1, :],
                     xl_d[rows, 0:ST].rearrange("(ho hp) s -> hp ho s",
                                                hp=128))
                if g == 1:
                    # only the st0 slice of the RoPE tables blocks A(0)
                    ldma(cost[:, 0:ST], cos_d[:, 0:ST])
                    ldma(sins[:, 0:ST], sin_d[:, 0:ST])
            for g in range(NG):
                rows = slice(g * HG * 128, (g + 1) * HG * 128)
                nc.sync.dma_start(
                    wt[g][:, :, 4 * 128:],
                    wt_d[rows, 4 * 128:].rearrange("(ho hp) o -> hp ho o",
                                                   hp=128))
            nc.sync.dma_start(tri[:], tri_d[:])
            issue_xt(1)
            nc.sync.dma_start(cost[:, ST:], cos_d[:, ST:])
            nc.sync.dma_start(sins[:, ST:], sin_d[:, ST:])
            nc.sync.dma_start(
                wot[:], wot_d.rearrange("(co cp) o -> cp co o", cp=128))

            # pre-zero the pt buffers once: diagonal-block left regions are
            # never written by the narrowed exp, only multiplied by tri=0
            for hh_ in range(HPC):
                ptz = ptpool.tile([128, 4 * NST, ST], F16, name=f"pth{hh_}")
                nc.gpsimd.memset(ptz[:], 0.0)

            # persistent per-head tensors
            qk = [[qpool.tile([D, ST], F16, name=f"qk{oc}_{st}")
                   for st in range(NST)] for oc in range(4)]
            v_sb = [qpool.tile([128, ST // 128, HPC * D], F16, name=f"v{st}")
                    for st in range(NST)]
            ctx = [[xpool.tile([D, 2, ST], FP8, name=f"ctx{h}_{t}")
                    for t in range(NST)] for h in range(HPC)]

            def a_qk(st, oc):
                ssl = slice(st * ST, (st + 1) * ST)
                xts = xt_tiles[st]
                ps = psA.tile([128, ST], F32)
                for hcc in range(HC):
                    g, ho = hcc // HG, hcc % HG
                    nc.tensor.matmul(
                        ps[:],
                        wt[g][:, ho, None,
                              oc * 128:(oc + 1) * 128].broadcast_to(
                                  (128, 2, 128)),
                        xts[:, hcc, :, :],
                        start=(hcc == 0), stop=(hcc == HC - 1),
                        perf_mode=DR)
                # RoPE (rotate-half sign folded into sins) -> qk f16.
                # Stage the psum through an Act fp16 copy so the DVE
                # multiplies run in 2x mode (shifts load from the saturated
                # DVE to the idler Act engine).
                dst = qk[oc][st]
                t2 = rpool.tile([128, ST], F16, name="t2")
                nc.vector.tensor_mul(t2[0:64, :], ps[64:128, :],
                                     sins[0:64, ssl])
                nc.vector.tensor_mul(t2[64:128, :], ps[0:64, :],
                                     sins[64:128, ssl])
                if st <= 1:
                    # startup only: stage ps through an idle-Act fp16 copy so
                    # the remaining DVE ops run 2x -- shortens the rope
                    # chains gating B(0,0) at the feed tail
                    qsb = rpool.tile([128, ST], F16, name="qsb")
                    nc.scalar.copy(qsb[:], ps[:])
                    nc.vector.tensor_mul(dst[:], qsb[:], cost[:, ssl])
                else:
                    nc.vector.tensor_mul(dst[:], ps[:], cost[:, ssl])
                nc.vector.tensor_add(dst[:], dst[:], t2[:])

            def a_v(st, scp):
                xts = xt_tiles[st]
                ps = psA.tile([128, 2, HPC * D], F32)
                for sc2 in range(2):
                    sc = scp * 2 + sc2
                    for hcc in range(HC):
                        g, ho = hcc // HG, hcc % HG
                        nc.tensor.matmul(
                            ps[:, sc2, :],
                            xts[:, hcc, :, sc * 128:(sc + 1) * 128],
                            wt[g][:, ho, None, 4 * 128:].broadcast_to(
                                (128, 2, HPC * D)),
                            start=(hcc == 0), stop=(hcc == HC - 1),
                            perf_mode=DR)
                nc.scalar.copy(v_sb[st][:, scp * 2:scp * 2 + 2, :], ps[:])

            def phaseB(t, h, cfill=(), afill=None):
                nkj = 4 * (t + 1)
                pt = ptpool.tile([128, nkj, ST], F16, name=f"pth{h}")
                # diagonal (masked) chunks first: their Pool mask latency then
                # overlaps later exps instead of sitting at the chain tail
                jlist = list(range(4 * t, nkj)) + list(range(0, 4 * t))
                for ji, j in enumerate(jlist):
                    if h == 0 and ji == max(0, min(6, nkj - 2)):
                        # fill the exp-paced S-window with this tile's own
                        # v projection (needs only xt + wt-v, and cp needs
                        # v only after the S-loop)
                        a_v(t, 0)
                        a_v(t, 1)
                    if cfill and ji == min(10, nkj - 1):
                        for (ct_, sc_) in cfill:
                            c_sc(ct_, sc_, dve_drains=(h == 0),
                                 act3=False)
                    if afill is not None and ji == nkj // 2:
                        for oc_ in afill[1]:
                            a_qk(afill[0], oc_)
                    off = 128 * j - ST * t
                    # diagonal blocks: columns qi < off are fully masked --
                    # skip them in scores matmul + exp (tri zeroes the
                    # pre-zeroed left region anyway)
                    w = ST - off if off > 0 else ST
                    sp = psS.tile([128, ST], F32, name="sp")
                    nc.tensor.matmul(
                        sp[:, 0:w],
                        qk[2 + h][j // 4][:, (j % 4) * 128:(j % 4 + 1) * 128],
                        qk[h][t][:, ST - w:],
                        start=True, stop=True)
                    # probs (unnormalized): exp(scores); score scale
                    # s_p^2/sqrt(D) lives in the RoPE tables
                    nc.scalar.activation(
                        pt[:, j, ST - w:], sp[:, 0:w],
                        mybir.ActivationFunctionType.Exp,
                        bias=0.0, scale=1.0)
                    if off >= 0:  # diagonal block: tril mask
                        nc.gpsimd.tensor_mul(
                            pt[:, j, :], pt[:, j, :],
                            tri[:, 384 - off:896 - off])
                # denominator first (recip overlaps ctx accumulation):
                # all-ones lhsT partition-reduce, bcast to all partitions.
                # h=0 pre-sums the chunks on DVE (fp16 2x) to save PE time;
                # h=1 keeps the full PE accumulation since its denominator
                # gates ctx -> o_proj.
                bp = psC.tile([128, ST], F32)
                if nkj > 1 and not (t == NST - 1 and h == 1):
                    # chain in jlist order: masked (diagonal) chunks first so
                    # the chain tail never waits on a Pool mask
                    pts = rpool.tile([128, ST], F16, name="pts")
                    nc.vector.tensor_add(pts[:], pt[:, jlist[0], :],
                                         pt[:, jlist[1], :])
                    for j in jlist[2:]:
                        nc.vector.tensor_add(pts[:], pts[:], pt[:, j, :])
                    nc.tensor.matmul(bp[:], osq[:], pts[:],
                                     start=True, stop=True)
                else:
                    for j in range(nkj):
                        nc.tensor.matmul(bp[:], osq[:], pt[:, j, :],
                                         start=(j == 0), stop=(j == nkj - 1))
                # ctx^T[d, qi] accumulate over kj
                cp = psC.tile([128, ST], F32)
                for ji, j in enumerate(jlist):
                    off = 128 * j - ST * t
                    w = ST - off if off > 0 else ST
                    nc.tensor.matmul(
                        cp[:, ST - w:],
                        v_sb[j // 4][:, j % 4, h * D:(h + 1) * D],
                        pt[:, j, ST - w:],
                        start=(ji == 0), stop=(ji == nkj - 1))
                rbp = rpool.tile([128, ST], F32, name="rbp")
                nc.vector.reciprocal(rbp[:], bp[:])
                tmp = rpool.tile([128, ST], F32, name="ctmp")
                nc.vector.tensor_mul(tmp[:], cp[:], rbp[:])
                # fp8 hi/lo split for the o_proj DoubleRow pair. Normally on
                # Pool (DVE is co-critical); for the final tile the chain
                # latency gates the endgame, so stay on DVE (no engine hops).
                ct = ctx[h][t]
                eng = nc.vector if h == 1 else nc.gpsimd
                eng.tensor_copy(ct[:, 0, :], tmp[:])
                eng.tensor_sub(ct[:, 1, :], tmp[:], ct[:, 0, :])

            def c_sc(t, sc, endgame=False, dve_drains=False, act3=True):
                row = (t * 4 + sc) * 128
                ob = opool.tile([128, H], F16)
                for ot in range(4):
                    if endgame:
                        # A/B phases are over: psS/psC banks are free, rotate
                        # po across all pools so drains pipeline wider
                        pool, nm = ((psO, "po"), (psS, "sp"), (psC, "bp"),
                                    (psC, "cp"))[(sc * 4 + ot) % 4]
                        po = pool.tile([128, ST], F32, name=nm)
                    else:
                        po = psO.tile([128, ST], F32, name="po")
                    for cc in range(HPC):
                        nc.tensor.matmul(
                            po[:],
                            ctx[cc][t][:, :, sc * 128:(sc + 1) * 128],
                            wot[:, cc, None,
                                ot * ST:(ot + 1) * ST].broadcast_to(
                                    (128, 2, ST)),
                            start=(cc == 0), stop=(cc == HPC - 1),
                            perf_mode=DR)
                    if (not dve_drains) and ((ot % 2 == 0) if (endgame or not act3) else (ot != 3)):
                        nc.scalar.copy(ob[:, ot * ST:(ot + 1) * ST], po[:])
                    else:
                        nc.vector.tensor_copy(ob[:, ot * ST:(ot + 1) * ST],
                                              po[:])
                    if t == NST - 1 and sc == 3:
                        # very last row block: stream half-stores so the
                        # final DMA is small
                        if ot % 2 == 1:
                            nc.sync.dma_start(
                                out_d[row:row + 128,
                                      (ot - 1) * ST:(ot + 1) * ST],
                                ob[:, (ot - 1) * ST:(ot + 1) * ST])
                if not (t == NST - 1 and sc == 3):
                    nc.sync.dma_start(out_d[row:row + 128, :], ob[:])

            def phaseA(st, cwork=(), act3=True, ocs=None):
                # interleave A(st) oc/v groups with deferred C sc groups so
                # DVE rope ops and psO drains alternate in the queues
                cs = list(cwork)
                # st0 (no C interleave): oc order 0,2,1,3 so B(0,h0)'s rope
                # gate (oc0+oc2 chains) clears two chains earlier
                order = ocs if ocs is not None else (
                    (0, 2, 1, 3) if st == 0 else (0, 1, 2, 3))
                for i, oc in enumerate(order):
                    a_qk(st, oc)
                    if i < len(cs):
                        c_sc(*cs[i], act3=act3)
                for i in range(len(order), len(cs)):
                    c_sc(*cs[i], act3=act3)
                if st + 2 < NST:
                    issue_xt(st + 2)

            # C-work is progressively deferred so the latency-bound t=3
            # region (B(3) exp pacing, ctx chains) has PE fill
            phaseA(0)
            phaseB(0, 0)
            phaseB(0, 1, afill=(1, (0, 2, 1, 3)))
            phaseA(1, cwork=[(0, 0), (0, 1), (0, 2), (0, 3)], ocs=())
            phaseB(1, 0)
            phaseB(1, 1, afill=(2, (0, 2, 1, 3)))
            phaseA(2, cwork=[(1, 0), (1, 1)], ocs=())
            phaseB(2, 0)
            phaseB(2, 1, afill=(3, (0, 2, 1, 3)))
            phaseA(3, cwork=[(1, 2), (1, 3), (2, 0)], act3=False, ocs=())
            phaseB(3, 0, cfill=[(2, 1), (2, 2)])
            phaseB(3, 1, cfill=[(2, 3)])
            for sc in range(4):
                c_sc(3, sc, endgame=True)

            if timing:
                nc.sync.dma_start(out_x[:], out_d[S - 128:, :])

    nc.compile()
    return nc


def _host_prep(hidden_states, w_proj, w_o):
    import ml_dtypes
    x = np.asarray(hidden_states, dtype=np.float32).reshape(S, H)
    w_proj = np.asarray(w_proj, dtype=np.float32)
    w_o = np.asarray(w_o, dtype=np.float32)

    # BitNet b1.58 per-tensor absmean quantization (ternary, scale factored
    # out: device weights are exactly {-1,0,1} in fp8)
    s_p = np.float32(np.mean(np.abs(w_proj), dtype=np.float32)) + np.float32(1e-5)
    s_o = np.float32(np.mean(np.abs(w_o), dtype=np.float32)) + np.float32(1e-5)
    tp = np.clip(np.round(w_proj / s_p), -1.0, 1.0).astype(np.float32)
    to = np.clip(np.round(w_o / s_o), -1.0, 1.0).astype(np.float32)

    xt = np.ascontiguousarray(x.T)                      # [H, S]
    xh = xt.astype(ml_dtypes.float8_e4m3)
    xl = (xt - xh.astype(np.float32)).astype(ml_dtypes.float8_e4m3)

    # RoPE tables, feature-major, rotate-half sign folded into sin
    inv_freq = (1.0 / (ROPE_BASE ** (np.arange(0, D, 2, dtype=np.float32) / D))
                ).astype(np.float32)
    t = np.arange(S, dtype=np.float32)
    freqs = np.outer(inv_freq, t).astype(np.float32)    # [64, S]
    # fold the score scale s_p^2/sqrt(D) into the tables (sqrt per side) so
    # the exp activation needs no per-partition scale operand
    f = np.float32(s_p / np.float32(D) ** 0.25)
    cosT = (np.concatenate([np.cos(freqs), np.cos(freqs)], 0) * f
            ).astype(np.float16)
    sinS = (np.concatenate([-np.sin(freqs), np.sin(freqs)], 0) * f
            ).astype(np.float16)

    # shifted tril mask bank: tri[p, x] = 1 if p <= x - 384
    p = np.arange(128)[:, None]
    xx = np.arange(896)[None, :]
    tri = (p <= xx - 384).astype(np.float16)

    in_maps = []
    for c in range(NCORES):
        r = slice(c * HPC * D, (c + 1) * HPC * D)       # 256 features
        wt_c = np.ascontiguousarray(np.concatenate(
            [tp[:H][r], tp[H:2 * H][r], tp[2 * H:][r]], 0).T
        ).astype(ml_dtypes.float8_e4m3)
        wot_c = np.ascontiguousarray(to[:, r].T).astype(ml_dtypes.float8_e4m3)
        in_maps.append({
            "xh": xh, "xl": xl, "wt": wt_c, "wot": wot_c,
            "cost": cosT, "sins": sinS, "tri": tri,
        })
    return in_maps, np.float32(s_p * s_o)


def kernel(hidden_states, attention_mask, w_proj, w_o):
    global _built
    if _built is None:
        _built = _build()
    nc = _built
    in_maps, osc = _host_prep(hidden_states, w_proj, w_o)
    res = run_bass_kernel_spmd(nc, in_maps, core_ids=list(range(NCORES)))
    acc = np.zeros((S, H), np.float32)
    for c in range(NCORES):
        acc += res.results[c]["out"].astype(np.float32)
    return (acc * osc).reshape(1, S, H)
